# revision 17
# baseline (speedup 1.0000x reference)
"""Trainium2 Bass kernel for a 3-layer L1Linear + monotone-pair-activation MLP.

Network (reference):
    ls = scale ** (1/3)
    h0 = monotone_act((X*ls) @ w0 + b0, act0)      w0 = wpn0[:2048] - wpn0[2048:]
    h1 = monotone_act((h0*ls) @ w1 + b1, act1)     w1 = wpn1[:2048] - wpn1[2048:]
    out = (h1*ls) @ w2 + b2                        w2 = wpn2[:2048] - wpn2[2048:]

monotone_act with arity=2, out_dim=1 reduces per pair (x0, x1) = (z[:,2g], z[:,2g+1]) to
    out_g = p3*min(x0,x1) + p2*relu(x1-x0) + p1*relu(x0-x1)
          = p3*x0 + (p1+p2-p3)*relu(d) - p2*d          with d = x0 - x1
so each layer's device matmul uses the column-rearranged weights
[W_even | W_even - W_odd]: block 0 produces x0 per group, block 1 produces d,
and the act is 4 fused per-partition-scalar element ops.

Device layout: activations are kept transposed [features, batch]; features live
on SBUF partitions so act coefficients are per-partition scalars.  Each core
handles 1024 batch columns (data parallel over 8 cores, weights replicated).
"""

import numpy as np
import ml_dtypes

P = 128
B_CORE = 1024          # batch columns per core
N_CORES = 8
KT = 16                # contraction tiles (2048 / 128)
MT01 = 32              # output m-tiles for layers 0/1 (4096 features)
MT2 = 16               # output m-tiles for layer 2 (2048 features)
NC2 = B_CORE // 512    # 512-wide psum chunks per core batch

_CACHE = {}


def _build_program():
    import concourse.mybir as mybir
    import concourse.tile as tile
    from concourse import bacc

    fp32 = mybir.dt.float32
    bf16 = mybir.dt.bfloat16
    MULT = mybir.AluOpType.mult
    ADD = mybir.AluOpType.add

    nc = bacc.Bacc(None, target_bir_lowering=False, debug=False)
    xt = nc.dram_tensor("xt", [KT, P, B_CORE], bf16, kind="ExternalInput").ap()
    w0 = nc.dram_tensor("w0t", [16, P, 2, KT, P], bf16, kind="ExternalInput").ap()
    w1 = nc.dram_tensor("w1t", [16, P, 2, KT, P], bf16, kind="ExternalInput").ap()
    w2 = nc.dram_tensor("w2t", [MT2, P, KT, P], bf16, kind="ExternalInput").ap()
    c0 = nc.dram_tensor("coef0", [P, 5, 16], fp32, kind="ExternalInput").ap()
    c1 = nc.dram_tensor("coef1", [P, 5, 16], fp32, kind="ExternalInput").ap()
    b2 = nc.dram_tensor("b2c", [P, 16], fp32, kind="ExternalInput").ap()
    out = nc.dram_tensor("out", [MT2 * P, B_CORE], fp32, kind="ExternalOutput").ap()

    # coefficient vector rows in the coef tensors
    V_P3, V_NEGP2, V_CR, V_SB, V_DB = range(5)

    with tile.TileContext(nc) as tc, \
         tc.tile_pool(name="constp", bufs=1) as constp, \
         tc.tile_pool(name="actsp", bufs=1) as actsp, \
         tc.tile_pool(name="wpool", bufs=4) as wpool, \
         tc.tile_pool(name="tpool", bufs=4) as tpool, \
         tc.tile_pool(name="opool", bufs=3) as opool, \
         tc.tile_pool(name="psum", bufs=8, space="PSUM") as psum:

        coef0_sb = constp.tile([P, 5, 16], fp32)
        nc.scalar.dma_start(out=coef0_sb, in_=c0)
        coef1_sb = constp.tile([P, 5, 16], fp32)
        nc.scalar.dma_start(out=coef1_sb, in_=c1)
        b2_sb = constp.tile([P, 16], fp32)
        nc.scalar.dma_start(out=b2_sb, in_=b2)

        def issue_pair(w_d, t, tag_id):
            s = wpool.tile([P, 2, KT, P], bf16, tag="w", name=f"w_{tag_id}_{t}")
            nc.sync.dma_start(out=s, in_=w_d[t])
            return s

        # Prefetch the first pair of layer-0 weight strips BEFORE the X load:
        # all bulk DMA drains FIFO through one HW queue, and the PE's first
        # matmul group gates on the x0-half of pair 0 plus X k-tile 0.
        wpair00 = wpool.tile([P, 2, KT, P], bf16, tag="w")
        nc.sync.dma_start(out=wpair00[:, 0], in_=w0[0, :, 0])
        x_sb = actsp.tile([P, KT, B_CORE], bf16, tag="x")
        for k in range(3):
            nc.scalar.dma_start(out=x_sb[:, k, :], in_=xt[k])
        nc.sync.dma_start(out=wpair00[:, 1], in_=w0[0, :, 1])
        for k in range(3, 6):
            nc.scalar.dma_start(out=x_sb[:, k, :], in_=xt[k])
        pf = {(0, 0): wpair00, (0, 1): issue_pair(w0, 1, 0)}
        for k in range(6, KT):
            nc.scalar.dma_start(out=x_sb[:, k, :], in_=xt[k])
        a0_sb = actsp.tile([P, KT, B_CORE], bf16, tag="a0")
        a1_sb = actsp.tile([P, KT, B_CORE], bf16, tag="a1")

        def layer01(lidx, w_d, coef_sb, rhs_sb, aout_sb):
            for t in range(16):
                wpair = pf.pop((lidx, t), None) or issue_pair(w_d, t, lidx)
                for c in range(NC2):
                    cs = slice(c * 512, (c + 1) * 512)
                    px = psum.tile([P, 512], fp32, tag="ps", name=f"px{c}")
                    pd = psum.tile([P, 512], fp32, tag="ps", name=f"pd{c}")
                    for ptile, h in ((px, 0), (pd, 1)):
                        for k in range(KT):
                            nc.tensor.matmul(
                                ptile,
                                wpair[:, h, k, :],
                                rhs_sb[:, k, cs],
                                start=(k == 0),
                                stop=(k == KT - 1),
                            )
                    r = tpool.tile([P, 512], fp32, tag="r")
                    nc.scalar.activation(
                        out=r, in_=pd,
                        func=mybir.ActivationFunctionType.Relu,
                        bias=coef_sb[:, V_DB, t:t + 1], scale=1.0,
                    )
                    t1 = tpool.tile([P, 512], fp32, tag="t1")
                    nc.vector.tensor_scalar(
                        out=t1, in0=r,
                        scalar1=coef_sb[:, V_CR, t:t + 1],
                        scalar2=coef_sb[:, V_SB, t:t + 1],
                        op0=MULT, op1=ADD,
                    )
                    t2 = tpool.tile([P, 512], fp32, tag="t2")
                    nc.vector.scalar_tensor_tensor(
                        out=t2, in0=pd,
                        scalar=coef_sb[:, V_NEGP2, t:t + 1], in1=t1,
                        op0=MULT, op1=ADD,
                    )
                    nc.vector.scalar_tensor_tensor(
                        out=aout_sb[:, t, cs], in0=px,
                        scalar=coef_sb[:, V_P3, t:t + 1], in1=t2,
                        op0=MULT, op1=ADD,
                    )

        layer01(0, w0, coef0_sb, x_sb, a0_sb)
        layer01(1, w1, coef1_sb, a0_sb, a1_sb)

        for t in range(MT2):
            wt = wpool.tile([P, KT, P], bf16, tag="w2", bufs=6, name=f"w2_{t}")
            nc.sync.dma_start(out=wt, in_=w2[t])
            o = opool.tile([P, B_CORE], fp32, tag="o")
            for c in range(NC2):
                cs = slice(c * 512, (c + 1) * 512)
                pz = psum.tile([P, 512], fp32, tag="ps")
                for k in range(KT):
                    nc.tensor.matmul(
                        pz,
                        wt[:, k, :],
                        a1_sb[:, k, cs],
                        start=(k == 0),
                        stop=(k == KT - 1),
                    )
                nc.vector.tensor_scalar_add(out=o[:, cs], in0=pz, scalar1=b2_sb[:, t:t + 1])
                nc.sync.dma_start(out=out[t * P:(t + 1) * P, cs], in_=o[:, cs])

    nc.compile()
    return nc


def _tile_w(w):
    """[K, M] fp32 -> [M/128, 128(ki), K/128, 128(mi)] bf16, contiguous."""
    K, M = w.shape
    wt = w.astype(ml_dtypes.bfloat16).reshape(K // P, P, M // P, P)
    return np.ascontiguousarray(wt.transpose(2, 1, 0, 3))


def _prep_layer01(wpn, b, act, ls):
    """Host prep for a pair-act layer: weights [2048,4096]->[x0|d] tiled,
    plus the 5 per-feature coefficient vectors packed [128, 5, 16]."""
    inw = wpn.shape[0] // 2
    w = (wpn[:inw].astype(np.float64) - wpn[inw:].astype(np.float64)) * ls
    wx = w[:, 0::2]
    wd = wx - w[:, 1::2]
    wcat = np.concatenate([wx, wd], axis=1).astype(np.float32)  # [2048, 4096]

    p1 = act[:, 1, 0].astype(np.float64)
    p2 = act[:, 2, 0].astype(np.float64)
    p3 = act[:, 3, 0].astype(np.float64)
    bx0 = b[0::2].astype(np.float64)
    db = bx0 - b[1::2].astype(np.float64)
    vecs = np.stack([
        p3,                 # V_P3
        -p2,                # V_NEGP2
        p1 + p2 - p3,       # V_CR
        p3 * bx0 - p2 * db,  # V_SB
        db,                 # V_DB
    ], axis=0).astype(np.float32)                                # [5, 2048]
    coef = np.ascontiguousarray(vecs.reshape(5, 16, P).transpose(2, 0, 1))
    tiled = _tile_w(wcat)                                        # [32, P, KT, P]
    paired = np.ascontiguousarray(np.stack([tiled[:16], tiled[16:]], axis=2))
    return paired, coef                                          # [16, P, 2, KT, P]


def kernel(X, scale, wpn0, b0, act0, wpn1, b1, act1, wpn2, b2):
    from concourse.bass_utils import run_bass_kernel_spmd

    X = np.asarray(X, dtype=np.float32)
    ls = float(np.float32(np.asarray(scale, dtype=np.float32) ** np.float32(1.0 / 3.0)))

    w0t, coef0 = _prep_layer01(np.asarray(wpn0), np.asarray(b0), np.asarray(act0), ls)
    w1t, coef1 = _prep_layer01(np.asarray(wpn1), np.asarray(b1), np.asarray(act1), ls)
    inw2 = wpn2.shape[0] // 2
    w2 = ((np.asarray(wpn2)[:inw2].astype(np.float64)
           - np.asarray(wpn2)[inw2:].astype(np.float64)) * ls).astype(np.float32)
    w2t = _tile_w(w2)
    b2c = np.ascontiguousarray(
        np.asarray(b2, dtype=np.float32).reshape(16, P).T)

    if "nc" not in _CACHE:
        _CACHE["nc"] = _build_program()
    nc = _CACHE["nc"]

    in_maps = []
    for c in range(N_CORES):
        xs = X[c * B_CORE:(c + 1) * B_CORE]            # [1024, 2048]
        xtc = np.ascontiguousarray(xs.T.astype(ml_dtypes.bfloat16)).reshape(KT, P, B_CORE)
        in_maps.append({
            "xt": xtc,
            "w0t": w0t, "w1t": w1t, "w2t": w2t,
            "coef0": coef0, "coef1": coef1, "b2c": b2c,
        })

    res = run_bass_kernel_spmd(nc, in_maps, core_ids=list(range(N_CORES)))
    return np.concatenate([r["out"].T for r in res.results], axis=0)


# revision 18
# speedup vs baseline: 1.0114x; 1.0114x over previous
"""Trainium2 Bass kernel for a 3-layer L1Linear + monotone-pair-activation MLP.

Network (reference):
    ls = scale ** (1/3)
    h0 = monotone_act((X*ls) @ w0 + b0, act0)      w0 = wpn0[:2048] - wpn0[2048:]
    h1 = monotone_act((h0*ls) @ w1 + b1, act1)     w1 = wpn1[:2048] - wpn1[2048:]
    out = (h1*ls) @ w2 + b2                        w2 = wpn2[:2048] - wpn2[2048:]

monotone_act with arity=2, out_dim=1 reduces per pair (x0, x1) = (z[:,2g], z[:,2g+1]) to
    out_g = p3*min(x0,x1) + p2*relu(x1-x0) + p1*relu(x0-x1)
          = p3*x0 + (p1+p2-p3)*relu(d) - p2*d          with d = x0 - x1
so each layer's device matmul uses the column-rearranged weights
[W_even | W_even - W_odd]: block 0 produces x0 per group, block 1 produces d,
and the act is 4 fused per-partition-scalar element ops.

Device layout: activations are kept transposed [features, batch]; features live
on SBUF partitions so act coefficients are per-partition scalars.  Each core
handles 1024 batch columns (data parallel over 8 cores, weights replicated).
"""

import numpy as np
import ml_dtypes

P = 128
B_CORE = 1024          # batch columns per core
N_CORES = 8
KT = 16                # contraction tiles (2048 / 128)
MT01 = 32              # output m-tiles for layers 0/1 (4096 features)
MT2 = 16               # output m-tiles for layer 2 (2048 features)
NC2 = B_CORE // 512    # 512-wide psum chunks per core batch

_CACHE = {}


def _build_program():
    import concourse.mybir as mybir
    import concourse.tile as tile
    from concourse import bacc

    fp32 = mybir.dt.float32
    bf16 = mybir.dt.bfloat16
    MULT = mybir.AluOpType.mult
    ADD = mybir.AluOpType.add

    nc = bacc.Bacc(None, target_bir_lowering=False, debug=False)
    xt = nc.dram_tensor("xt", [KT, P, B_CORE], bf16, kind="ExternalInput").ap()
    w0 = nc.dram_tensor("w0t", [16, P, 2, KT, P], bf16, kind="ExternalInput").ap()
    w1 = nc.dram_tensor("w1t", [16, P, 2, KT, P], bf16, kind="ExternalInput").ap()
    w2 = nc.dram_tensor("w2t", [MT2, P, KT, P], bf16, kind="ExternalInput").ap()
    c0 = nc.dram_tensor("coef0", [P, 5, 16], fp32, kind="ExternalInput").ap()
    c1 = nc.dram_tensor("coef1", [P, 5, 16], fp32, kind="ExternalInput").ap()
    b2 = nc.dram_tensor("b2c", [P, 16], fp32, kind="ExternalInput").ap()
    out = nc.dram_tensor("out", [MT2 * P, B_CORE], fp32, kind="ExternalOutput").ap()

    # coefficient vector rows in the coef tensors
    V_P3, V_NEGP2, V_CR, V_SB, V_DB = range(5)

    with tile.TileContext(nc) as tc, \
         tc.tile_pool(name="constp", bufs=1) as constp, \
         tc.tile_pool(name="actsp", bufs=1) as actsp, \
         tc.tile_pool(name="wpool", bufs=4) as wpool, \
         tc.tile_pool(name="tpool", bufs=4) as tpool, \
         tc.tile_pool(name="opool", bufs=3) as opool, \
         tc.tile_pool(name="psum", bufs=8, space="PSUM") as psum:

        coef0_sb = constp.tile([P, 5, 16], fp32)
        nc.sync.dma_start(out=coef0_sb, in_=c0)
        coef1_sb = constp.tile([P, 5, 16], fp32)
        nc.sync.dma_start(out=coef1_sb, in_=c1)
        b2_sb = constp.tile([P, 16], fp32)
        nc.sync.dma_start(out=b2_sb, in_=b2)

        def issue_pair(w_d, t, tag_id):
            s = wpool.tile([P, 2, KT, P], bf16, tag="w", name=f"w_{tag_id}_{t}")
            nc.sync.dma_start(out=s, in_=w_d[t])
            return s

        # Prefetch the first pair of layer-0 weight strips BEFORE the X load:
        # all bulk DMA drains FIFO through one HW queue, and the PE's first
        # matmul group gates on the x0-half of pair 0 plus X k-tile 0.
        wpair00 = wpool.tile([P, 2, KT, P], bf16, tag="w")
        nc.sync.dma_start(out=wpair00[:, 0], in_=w0[0, :, 0])
        x_sb = actsp.tile([P, KT, B_CORE], bf16, tag="x")
        for k in range(3):
            nc.sync.dma_start(out=x_sb[:, k, :], in_=xt[k])
        nc.sync.dma_start(out=wpair00[:, 1], in_=w0[0, :, 1])
        for k in range(3, 6):
            nc.sync.dma_start(out=x_sb[:, k, :], in_=xt[k])
        pf = {(0, 0): wpair00, (0, 1): issue_pair(w0, 1, 0)}
        for k in range(6, KT):
            nc.sync.dma_start(out=x_sb[:, k, :], in_=xt[k])
        a0_sb = actsp.tile([P, KT, B_CORE], bf16, tag="a0")
        a1_sb = actsp.tile([P, KT, B_CORE], bf16, tag="a1")

        def layer01(lidx, w_d, coef_sb, rhs_sb, aout_sb):
            for t in range(16):
                wpair = pf.pop((lidx, t), None) or issue_pair(w_d, t, lidx)
                for c in range(NC2):
                    cs = slice(c * 512, (c + 1) * 512)
                    px = psum.tile([P, 512], fp32, tag="ps", name=f"px{c}")
                    pd = psum.tile([P, 512], fp32, tag="ps", name=f"pd{c}")
                    for ptile, h in ((px, 0), (pd, 1)):
                        for k in range(KT):
                            nc.tensor.matmul(
                                ptile,
                                wpair[:, h, k, :],
                                rhs_sb[:, k, cs],
                                start=(k == 0),
                                stop=(k == KT - 1),
                            )
                    r = tpool.tile([P, 512], fp32, tag="r")
                    nc.scalar.activation(
                        out=r, in_=pd,
                        func=mybir.ActivationFunctionType.Relu,
                        bias=coef_sb[:, V_DB, t:t + 1], scale=1.0,
                    )
                    t1 = tpool.tile([P, 512], fp32, tag="t1")
                    nc.vector.tensor_scalar(
                        out=t1, in0=r,
                        scalar1=coef_sb[:, V_CR, t:t + 1],
                        scalar2=coef_sb[:, V_SB, t:t + 1],
                        op0=MULT, op1=ADD,
                    )
                    t2 = tpool.tile([P, 512], fp32, tag="t2")
                    nc.vector.scalar_tensor_tensor(
                        out=t2, in0=pd,
                        scalar=coef_sb[:, V_NEGP2, t:t + 1], in1=t1,
                        op0=MULT, op1=ADD,
                    )
                    nc.vector.scalar_tensor_tensor(
                        out=aout_sb[:, t, cs], in0=px,
                        scalar=coef_sb[:, V_P3, t:t + 1], in1=t2,
                        op0=MULT, op1=ADD,
                    )

        layer01(0, w0, coef0_sb, x_sb, a0_sb)
        layer01(1, w1, coef1_sb, a0_sb, a1_sb)

        for t in range(MT2):
            wt = wpool.tile([P, KT, P], bf16, tag="w2", bufs=6, name=f"w2_{t}")
            nc.sync.dma_start(out=wt, in_=w2[t])
            o = opool.tile([P, B_CORE], fp32, tag="o")
            for c in range(NC2):
                cs = slice(c * 512, (c + 1) * 512)
                pz = psum.tile([P, 512], fp32, tag="ps")
                for k in range(KT):
                    nc.tensor.matmul(
                        pz,
                        wt[:, k, :],
                        a1_sb[:, k, cs],
                        start=(k == 0),
                        stop=(k == KT - 1),
                    )
                nc.vector.tensor_scalar_add(out=o[:, cs], in0=pz, scalar1=b2_sb[:, t:t + 1])
                nc.sync.dma_start(out=out[t * P:(t + 1) * P, cs], in_=o[:, cs])

    nc.compile()
    return nc


def _tile_w(w):
    """[K, M] fp32 -> [M/128, 128(ki), K/128, 128(mi)] bf16, contiguous."""
    K, M = w.shape
    wt = w.astype(ml_dtypes.bfloat16).reshape(K // P, P, M // P, P)
    return np.ascontiguousarray(wt.transpose(2, 1, 0, 3))


def _prep_layer01(wpn, b, act, ls):
    """Host prep for a pair-act layer: weights [2048,4096]->[x0|d] tiled,
    plus the 5 per-feature coefficient vectors packed [128, 5, 16]."""
    inw = wpn.shape[0] // 2
    w = (wpn[:inw].astype(np.float64) - wpn[inw:].astype(np.float64)) * ls
    wx = w[:, 0::2]
    wd = wx - w[:, 1::2]
    wcat = np.concatenate([wx, wd], axis=1).astype(np.float32)  # [2048, 4096]

    p1 = act[:, 1, 0].astype(np.float64)
    p2 = act[:, 2, 0].astype(np.float64)
    p3 = act[:, 3, 0].astype(np.float64)
    bx0 = b[0::2].astype(np.float64)
    db = bx0 - b[1::2].astype(np.float64)
    vecs = np.stack([
        p3,                 # V_P3
        -p2,                # V_NEGP2
        p1 + p2 - p3,       # V_CR
        p3 * bx0 - p2 * db,  # V_SB
        db,                 # V_DB
    ], axis=0).astype(np.float32)                                # [5, 2048]
    coef = np.ascontiguousarray(vecs.reshape(5, 16, P).transpose(2, 0, 1))
    tiled = _tile_w(wcat)                                        # [32, P, KT, P]
    paired = np.ascontiguousarray(np.stack([tiled[:16], tiled[16:]], axis=2))
    return paired, coef                                          # [16, P, 2, KT, P]


def kernel(X, scale, wpn0, b0, act0, wpn1, b1, act1, wpn2, b2):
    from concourse.bass_utils import run_bass_kernel_spmd

    X = np.asarray(X, dtype=np.float32)
    ls = float(np.float32(np.asarray(scale, dtype=np.float32) ** np.float32(1.0 / 3.0)))

    w0t, coef0 = _prep_layer01(np.asarray(wpn0), np.asarray(b0), np.asarray(act0), ls)
    w1t, coef1 = _prep_layer01(np.asarray(wpn1), np.asarray(b1), np.asarray(act1), ls)
    inw2 = wpn2.shape[0] // 2
    w2 = ((np.asarray(wpn2)[:inw2].astype(np.float64)
           - np.asarray(wpn2)[inw2:].astype(np.float64)) * ls).astype(np.float32)
    w2t = _tile_w(w2)
    b2c = np.ascontiguousarray(
        np.asarray(b2, dtype=np.float32).reshape(16, P).T)

    if "nc" not in _CACHE:
        _CACHE["nc"] = _build_program()
    nc = _CACHE["nc"]

    in_maps = []
    for c in range(N_CORES):
        xs = X[c * B_CORE:(c + 1) * B_CORE]            # [1024, 2048]
        xtc = np.ascontiguousarray(xs.T.astype(ml_dtypes.bfloat16)).reshape(KT, P, B_CORE)
        in_maps.append({
            "xt": xtc,
            "w0t": w0t, "w1t": w1t, "w2t": w2t,
            "coef0": coef0, "coef1": coef1, "b2c": b2c,
        })

    res = run_bass_kernel_spmd(nc, in_maps, core_ids=list(range(N_CORES)))
    return np.concatenate([r["out"].T for r in res.results], axis=0)


# revision 19
# speedup vs baseline: 1.0128x; 1.0014x over previous
"""Trainium2 Bass kernel for a 3-layer L1Linear + monotone-pair-activation MLP.

Network (reference):
    ls = scale ** (1/3)
    h0 = monotone_act((X*ls) @ w0 + b0, act0)      w0 = wpn0[:2048] - wpn0[2048:]
    h1 = monotone_act((h0*ls) @ w1 + b1, act1)     w1 = wpn1[:2048] - wpn1[2048:]
    out = (h1*ls) @ w2 + b2                        w2 = wpn2[:2048] - wpn2[2048:]

monotone_act with arity=2, out_dim=1 reduces per pair (x0, x1) = (z[:,2g], z[:,2g+1]) to
    out_g = p3*min(x0,x1) + p2*relu(x1-x0) + p1*relu(x0-x1)
          = p3*x0 + (p1+p2-p3)*relu(d) - p2*d          with d = x0 - x1
so each layer's device matmul uses the column-rearranged weights
[W_even | W_even - W_odd]: block 0 produces x0 per group, block 1 produces d,
and the act is 4 fused per-partition-scalar element ops.

Device layout: activations are kept transposed [features, batch]; features live
on SBUF partitions so act coefficients are per-partition scalars.  Each core
handles 1024 batch columns (data parallel over 8 cores, weights replicated).
"""

import numpy as np
import ml_dtypes

P = 128
B_CORE = 1024          # batch columns per core
N_CORES = 8
KT = 16                # contraction tiles (2048 / 128)
MT01 = 32              # output m-tiles for layers 0/1 (4096 features)
MT2 = 16               # output m-tiles for layer 2 (2048 features)
NC2 = B_CORE // 512    # 512-wide psum chunks per core batch

_CACHE = {}


def _build_program():
    import concourse.mybir as mybir
    import concourse.tile as tile
    from concourse import bacc

    fp32 = mybir.dt.float32
    f16 = mybir.dt.float16
    MULT = mybir.AluOpType.mult
    ADD = mybir.AluOpType.add

    nc = bacc.Bacc(None, target_bir_lowering=False, debug=False)
    xt = nc.dram_tensor("xt", [KT, P, B_CORE], f16, kind="ExternalInput").ap()
    w0 = nc.dram_tensor("w0t", [16, P, 2, KT, P], f16, kind="ExternalInput").ap()
    w1 = nc.dram_tensor("w1t", [16, P, 2, KT, P], f16, kind="ExternalInput").ap()
    w2 = nc.dram_tensor("w2t", [MT2, P, KT, P], f16, kind="ExternalInput").ap()
    c0 = nc.dram_tensor("coef0", [P, 5, 16], fp32, kind="ExternalInput").ap()
    c1 = nc.dram_tensor("coef1", [P, 5, 16], fp32, kind="ExternalInput").ap()
    b2 = nc.dram_tensor("b2c", [P, 16], fp32, kind="ExternalInput").ap()
    out = nc.dram_tensor("out", [MT2 * P, B_CORE], fp32, kind="ExternalOutput").ap()

    # coefficient vector rows in the coef tensors
    V_P3, V_NEGP2, V_CR, V_SB, V_DB = range(5)

    with tile.TileContext(nc) as tc, \
         tc.tile_pool(name="constp", bufs=1) as constp, \
         tc.tile_pool(name="actsp", bufs=1) as actsp, \
         tc.tile_pool(name="wpool", bufs=4) as wpool, \
         tc.tile_pool(name="tpool", bufs=4) as tpool, \
         tc.tile_pool(name="opool", bufs=3) as opool, \
         tc.tile_pool(name="psum", bufs=8, space="PSUM") as psum:

        coef0_sb = constp.tile([P, 5, 16], fp32)
        nc.sync.dma_start(out=coef0_sb, in_=c0)
        coef1_sb = constp.tile([P, 5, 16], fp32)
        nc.sync.dma_start(out=coef1_sb, in_=c1)
        b2_sb = constp.tile([P, 16], fp32)
        nc.sync.dma_start(out=b2_sb, in_=b2)

        def issue_pair(w_d, t, tag_id):
            s = wpool.tile([P, 2, KT, P], f16, tag="w", name=f"w_{tag_id}_{t}")
            nc.sync.dma_start(out=s, in_=w_d[t])
            return s

        # Prefetch the first pair of layer-0 weight strips BEFORE the X load:
        # all bulk DMA drains FIFO through one HW queue, and the PE's first
        # matmul group gates on the x0-half of pair 0 plus X k-tile 0.
        wpair00 = wpool.tile([P, 2, KT, P], f16, tag="w")
        nc.sync.dma_start(out=wpair00[:, 0], in_=w0[0, :, 0])
        x_sb = actsp.tile([P, KT, B_CORE], f16, tag="x")
        for k in range(3):
            nc.sync.dma_start(out=x_sb[:, k, :], in_=xt[k])
        nc.sync.dma_start(out=wpair00[:, 1], in_=w0[0, :, 1])
        for k in range(3, 6):
            nc.sync.dma_start(out=x_sb[:, k, :], in_=xt[k])
        pf = {(0, 0): wpair00, (0, 1): issue_pair(w0, 1, 0)}
        for k in range(6, KT):
            nc.sync.dma_start(out=x_sb[:, k, :], in_=xt[k])
        a0_sb = actsp.tile([P, KT, B_CORE], f16, tag="a0")
        a1_sb = actsp.tile([P, KT, B_CORE], f16, tag="a1")

        def layer01(lidx, w_d, coef_sb, rhs_sb, aout_sb):
            for t in range(16):
                wpair = pf.pop((lidx, t), None) or issue_pair(w_d, t, lidx)
                for c in range(NC2):
                    cs = slice(c * 512, (c + 1) * 512)
                    px = psum.tile([P, 512], fp32, tag="ps", name=f"px{c}")
                    pd = psum.tile([P, 512], fp32, tag="ps", name=f"pd{c}")
                    for ptile, h in ((px, 0), (pd, 1)):
                        for k in range(KT):
                            nc.tensor.matmul(
                                ptile,
                                wpair[:, h, k, :],
                                rhs_sb[:, k, cs],
                                start=(k == 0),
                                stop=(k == KT - 1),
                            )
                    r = tpool.tile([P, 512], fp32, tag="r")
                    nc.scalar.activation(
                        out=r, in_=pd,
                        func=mybir.ActivationFunctionType.Relu,
                        bias=coef_sb[:, V_DB, t:t + 1], scale=1.0,
                    )
                    t1 = tpool.tile([P, 512], fp32, tag="t1")
                    nc.vector.tensor_scalar(
                        out=t1, in0=r,
                        scalar1=coef_sb[:, V_CR, t:t + 1],
                        scalar2=coef_sb[:, V_SB, t:t + 1],
                        op0=MULT, op1=ADD,
                    )
                    t2 = tpool.tile([P, 512], fp32, tag="t2")
                    nc.vector.scalar_tensor_tensor(
                        out=t2, in0=pd,
                        scalar=coef_sb[:, V_NEGP2, t:t + 1], in1=t1,
                        op0=MULT, op1=ADD,
                    )
                    nc.vector.scalar_tensor_tensor(
                        out=aout_sb[:, t, cs], in0=px,
                        scalar=coef_sb[:, V_P3, t:t + 1], in1=t2,
                        op0=MULT, op1=ADD,
                    )

        layer01(0, w0, coef0_sb, x_sb, a0_sb)
        layer01(1, w1, coef1_sb, a0_sb, a1_sb)

        for t in range(MT2):
            wt = wpool.tile([P, KT, P], f16, tag="w2", bufs=6, name=f"w2_{t}")
            nc.sync.dma_start(out=wt, in_=w2[t])
            o = opool.tile([P, B_CORE], fp32, tag="o")
            for c in range(NC2):
                cs = slice(c * 512, (c + 1) * 512)
                pz = psum.tile([P, 512], fp32, tag="ps")
                for k in range(KT):
                    nc.tensor.matmul(
                        pz,
                        wt[:, k, :],
                        a1_sb[:, k, cs],
                        start=(k == 0),
                        stop=(k == KT - 1),
                    )
                nc.vector.tensor_scalar(out=o[:, cs], in0=pz,
                                        scalar1=1.0 / WSCALE, scalar2=b2_sb[:, t:t + 1],
                                        op0=MULT, op1=ADD)
                nc.sync.dma_start(out=out[t * P:(t + 1) * P, cs], in_=o[:, cs])

    nc.compile()
    return nc


WSCALE = 256.0   # weights are scaled into fp16's normal range; act coefs unscale


def _tile_w(w):
    """[K, M] fp64 -> [M/128, 128(ki), K/128, 128(mi)] fp16, contiguous.
    Applies WSCALE so simplex-sized weights sit in fp16 normal range."""
    K, M = w.shape
    wt = (w * WSCALE).astype(np.float16).reshape(K // P, P, M // P, P)
    return np.ascontiguousarray(wt.transpose(2, 1, 0, 3))


def _prep_layer01(wpn, b, act, ls):
    """Host prep for a pair-act layer: weights [2048,4096]->[x0|d] tiled,
    plus the 5 per-feature coefficient vectors packed [128, 5, 16]."""
    inw = wpn.shape[0] // 2
    w = (wpn[:inw].astype(np.float64) - wpn[inw:].astype(np.float64)) * ls
    wx = w[:, 0::2]
    wd = wx - w[:, 1::2]
    wcat = np.concatenate([wx, wd], axis=1)        # [2048, 4096] fp64

    p1 = act[:, 1, 0].astype(np.float64)
    p2 = act[:, 2, 0].astype(np.float64)
    p3 = act[:, 3, 0].astype(np.float64)
    bx0 = b[0::2].astype(np.float64)
    db = bx0 - b[1::2].astype(np.float64)
    vecs = np.stack([
        p3 / WSCALE,         # V_P3     (psum z is WSCALE * true z)
        -p2 / WSCALE,        # V_NEGP2
        (p1 + p2 - p3) / WSCALE,  # V_CR
        p3 * bx0 - p2 * db,  # V_SB   (true-scale affine constant)
        db * WSCALE,         # V_DB   (relu bias applied at psum scale)
    ], axis=0).astype(np.float32)                                # [5, 2048]
    coef = np.ascontiguousarray(vecs.reshape(5, 16, P).transpose(2, 0, 1))
    tiled = _tile_w(wcat)                                        # [32, P, KT, P]
    paired = np.ascontiguousarray(np.stack([tiled[:16], tiled[16:]], axis=2))
    return paired, coef                                          # [16, P, 2, KT, P]


def kernel(X, scale, wpn0, b0, act0, wpn1, b1, act1, wpn2, b2):
    from concourse.bass_utils import run_bass_kernel_spmd

    X = np.asarray(X, dtype=np.float32)
    ls = float(np.float32(np.asarray(scale, dtype=np.float32) ** np.float32(1.0 / 3.0)))

    w0t, coef0 = _prep_layer01(np.asarray(wpn0), np.asarray(b0), np.asarray(act0), ls)
    w1t, coef1 = _prep_layer01(np.asarray(wpn1), np.asarray(b1), np.asarray(act1), ls)
    inw2 = wpn2.shape[0] // 2
    w2 = (np.asarray(wpn2)[:inw2].astype(np.float64)
          - np.asarray(wpn2)[inw2:].astype(np.float64)) * ls
    w2t = _tile_w(w2)
    b2c = np.ascontiguousarray(
        np.asarray(b2, dtype=np.float32).reshape(16, P).T)

    if "nc" not in _CACHE:
        _CACHE["nc"] = _build_program()
    nc = _CACHE["nc"]

    in_maps = []
    for c in range(N_CORES):
        xs = X[c * B_CORE:(c + 1) * B_CORE]            # [1024, 2048]
        xtc = np.ascontiguousarray(xs.T.astype(np.float16)).reshape(KT, P, B_CORE)
        in_maps.append({
            "xt": xtc,
            "w0t": w0t, "w1t": w1t, "w2t": w2t,
            "coef0": coef0, "coef1": coef1, "b2c": b2c,
        })

    res = run_bass_kernel_spmd(nc, in_maps, core_ids=list(range(N_CORES)))
    return np.concatenate([r["out"].T for r in res.results], axis=0)


# revision 22
# speedup vs baseline: 1.0156x; 1.0028x over previous
"""Trainium2 Bass kernel for a 3-layer L1Linear + monotone-pair-activation MLP.

Network (reference):
    ls = scale ** (1/3)
    h0 = monotone_act((X*ls) @ w0 + b0, act0)      w0 = wpn0[:2048] - wpn0[2048:]
    h1 = monotone_act((h0*ls) @ w1 + b1, act1)     w1 = wpn1[:2048] - wpn1[2048:]
    out = (h1*ls) @ w2 + b2                        w2 = wpn2[:2048] - wpn2[2048:]

monotone_act with arity=2, out_dim=1 reduces per pair (x0, x1) = (z[:,2g], z[:,2g+1]) to
    out_g = p3*min(x0,x1) + p2*relu(x1-x0) + p1*relu(x0-x1)
          = p3*x0 + (p1+p2-p3)*relu(d) - p2*d          with d = x0 - x1
so each layer's device matmul uses the column-rearranged weights
[W_even | W_even - W_odd]: block 0 produces x0 per group, block 1 produces d,
and the act is 4 fused per-partition-scalar element ops.

Device layout: activations are kept transposed [features, batch]; features live
on SBUF partitions so act coefficients are per-partition scalars.  Each core
handles 1024 batch columns (data parallel over 8 cores, weights replicated).
"""

import numpy as np
import ml_dtypes

P = 128
B_CORE = 1024          # batch columns per core
N_CORES = 8
KT = 16                # contraction tiles (2048 / 128)
MT01 = 32              # output m-tiles for layers 0/1 (4096 features)
MT2 = 16               # output m-tiles for layer 2 (2048 features)
NC2 = B_CORE // 512    # 512-wide psum chunks per core batch

_CACHE = {}


def _build_program():
    import concourse.mybir as mybir
    import concourse.tile as tile
    from concourse import bacc

    fp32 = mybir.dt.float32
    f16 = mybir.dt.float16
    MULT = mybir.AluOpType.mult
    ADD = mybir.AluOpType.add

    nc = bacc.Bacc(None, target_bir_lowering=False, debug=False)
    xt = nc.dram_tensor("xt", [KT, P, B_CORE], f16, kind="ExternalInput").ap()
    w0 = nc.dram_tensor("w0t", [16, P, 2, KT, P], f16, kind="ExternalInput").ap()
    w1 = nc.dram_tensor("w1t", [16, P, 2, KT, P], f16, kind="ExternalInput").ap()
    w2 = nc.dram_tensor("w2t", [MT2, P, KT, P], f16, kind="ExternalInput").ap()
    c0 = nc.dram_tensor("coef0", [P, 5, 16], fp32, kind="ExternalInput").ap()
    c1 = nc.dram_tensor("coef1", [P, 5, 16], fp32, kind="ExternalInput").ap()
    b2 = nc.dram_tensor("b2c", [P, 16], fp32, kind="ExternalInput").ap()
    out = nc.dram_tensor("out", [MT2 * P, B_CORE], fp32, kind="ExternalOutput").ap()

    # coefficient vector rows in the coef tensors
    V_P3, V_NEGP2, V_CR, V_SB, V_DB = range(5)

    with tile.TileContext(nc) as tc, \
         tc.tile_pool(name="constp", bufs=1) as constp, \
         tc.tile_pool(name="actsp", bufs=1) as actsp, \
         tc.tile_pool(name="wpool", bufs=4) as wpool, \
         tc.tile_pool(name="tpool", bufs=4) as tpool, \
         tc.tile_pool(name="opool", bufs=3) as opool, \
         tc.tile_pool(name="psum", bufs=8, space="PSUM") as psum:

        coef0_sb = constp.tile([P, 5, 16], fp32)
        nc.sync.dma_start(out=coef0_sb, in_=c0)
        coef1_sb = constp.tile([P, 5, 16], fp32)
        nc.sync.dma_start(out=coef1_sb, in_=c1)
        b2_sb = constp.tile([P, 16], fp32)
        nc.sync.dma_start(out=b2_sb, in_=b2)

        def issue_pair(w_d, t, tag_id):
            s = wpool.tile([P, 2, KT, P], f16, tag="w", name=f"w_{tag_id}_{t}")
            nc.sync.dma_start(out=s, in_=w_d[t])
            return s

        # Prefetch the first pair of layer-0 weight strips BEFORE the X load:
        # all bulk DMA drains FIFO through one HW queue, and the PE's first
        # matmul group gates on the x0-half of pair 0 plus X k-tile 0.
        wpair00 = wpool.tile([P, 2, KT, P], f16, tag="w")
        nc.sync.dma_start(out=wpair00[:, 0], in_=w0[0, :, 0])
        x_sb = actsp.tile([P, KT, B_CORE], f16, tag="x")
        for k in range(3):
            nc.sync.dma_start(out=x_sb[:, k, :], in_=xt[k])
        nc.sync.dma_start(out=wpair00[:, 1], in_=w0[0, :, 1])
        for k in range(3, 6):
            nc.sync.dma_start(out=x_sb[:, k, :], in_=xt[k])
        pf = {(0, 0): wpair00, (0, 1): issue_pair(w0, 1, 0)}
        for k in range(6, KT):
            nc.sync.dma_start(out=x_sb[:, k, :], in_=xt[k])
        a0_sb = actsp.tile([P, KT, B_CORE], f16, tag="a0")
        a1_sb = actsp.tile([P, KT, B_CORE], f16, tag="a1")

        def layer01(lidx, w_d, coef_sb, rhs_sb, aout_sb):
            for t in range(16):
                wpair = pf.pop((lidx, t), None) or issue_pair(w_d, t, lidx)
                for c in range(NC2):
                    cs = slice(c * 512, (c + 1) * 512)
                    px = psum.tile([P, 512], fp32, tag="ps", name=f"px{c}")
                    pd = psum.tile([P, 512], fp32, tag="ps", name=f"pd{c}")
                    for ptile, h in ((px, 0), (pd, 1)):
                        for k in range(KT):
                            nc.tensor.matmul(
                                ptile,
                                wpair[:, h, k, :],
                                rhs_sb[:, k, cs],
                                start=(k == 0),
                                stop=(k == KT - 1),
                            )
                    r = tpool.tile([P, 512], fp32, tag="r")
                    nc.scalar.activation(
                        out=r, in_=pd,
                        func=mybir.ActivationFunctionType.Relu,
                        bias=coef_sb[:, V_DB, t:t + 1], scale=1.0,
                    )
                    t1 = tpool.tile([P, 512], fp32, tag="t1")
                    nc.vector.tensor_scalar(
                        out=t1, in0=r,
                        scalar1=coef_sb[:, V_CR, t:t + 1],
                        scalar2=coef_sb[:, V_SB, t:t + 1],
                        op0=MULT, op1=ADD,
                    )
                    t2 = tpool.tile([P, 512], fp32, tag="t2")
                    nc.vector.scalar_tensor_tensor(
                        out=t2, in0=pd,
                        scalar=coef_sb[:, V_NEGP2, t:t + 1], in1=t1,
                        op0=MULT, op1=ADD,
                    )
                    nc.vector.scalar_tensor_tensor(
                        out=aout_sb[:, t, cs], in0=px,
                        scalar=coef_sb[:, V_P3, t:t + 1], in1=t2,
                        op0=MULT, op1=ADD,
                    )

        layer01(0, w0, coef0_sb, x_sb, a0_sb)
        layer01(1, w1, coef1_sb, a0_sb, a1_sb)

        for t in range(MT2):
            wt = wpool.tile([P, KT, P], f16, tag="w2", bufs=6, name=f"w2_{t}")
            nc.sync.dma_start(out=wt, in_=w2[t])
            o = opool.tile([P, B_CORE], fp32, tag="o")
            for c in range(NC2):
                cs = slice(c * 512, (c + 1) * 512)
                pz = psum.tile([P, 512], fp32, tag="ps")
                for k in range(KT):
                    nc.tensor.matmul(
                        pz,
                        wt[:, k, :],
                        a1_sb[:, k, cs],
                        start=(k == 0),
                        stop=(k == KT - 1),
                    )
                nc.vector.tensor_scalar(out=o[:, cs], in0=pz,
                                        scalar1=1.0 / WSCALE, scalar2=b2_sb[:, t:t + 1],
                                        op0=MULT, op1=ADD)
                nc.sync.dma_start(out=out[t * P:(t + 1) * P, cs], in_=o[:, cs])

    nc.compile()
    return nc


WSCALE = 256.0   # weights are scaled into fp16's normal range; act coefs unscale


def _tile_w(w):
    """[K, M] fp64 -> [M/128, 128(ki), K/128, 128(mi)] fp16, contiguous.
    Applies WSCALE so simplex-sized weights sit in fp16 normal range."""
    K, M = w.shape
    wt = (w * WSCALE).astype(np.float16).reshape(K // P, P, M // P, P)
    return np.ascontiguousarray(wt.transpose(2, 1, 0, 3))


def _prep_layer01(wpn, b, act, ls):
    """Host prep for a pair-act layer: weights [2048,4096]->[x0|d] tiled,
    plus the 5 per-feature coefficient vectors packed [128, 5, 16]."""
    inw = wpn.shape[0] // 2
    w = (wpn[:inw].astype(np.float64) - wpn[inw:].astype(np.float64)) * ls
    wx = w[:, 0::2]
    wd = wx - w[:, 1::2]
    wcat = np.concatenate([wx, wd], axis=1)        # [2048, 4096] fp64

    p1 = act[:, 1, 0].astype(np.float64)
    p2 = act[:, 2, 0].astype(np.float64)
    p3 = act[:, 3, 0].astype(np.float64)
    bx0 = b[0::2].astype(np.float64)
    db = bx0 - b[1::2].astype(np.float64)
    vecs = np.stack([
        p3 / WSCALE,         # V_P3     (psum z is WSCALE * true z)
        -p2 / WSCALE,        # V_NEGP2
        (p1 + p2 - p3) / WSCALE,  # V_CR
        p3 * bx0 - p2 * db,  # V_SB   (true-scale affine constant)
        db * WSCALE,         # V_DB   (relu bias applied at psum scale)
    ], axis=0).astype(np.float32)                                # [5, 2048]
    coef = np.ascontiguousarray(vecs.reshape(5, 16, P).transpose(2, 0, 1))
    tiled = _tile_w(wcat)                                        # [32, P, KT, P]
    paired = np.ascontiguousarray(np.stack([tiled[:16], tiled[16:]], axis=2))
    return paired, coef                                          # [16, P, 2, KT, P]


def kernel(X, scale, wpn0, b0, act0, wpn1, b1, act1, wpn2, b2):
    from concourse.bass_utils import run_bass_kernel_spmd

    X = np.asarray(X, dtype=np.float32)
    ls = float(np.float32(np.asarray(scale, dtype=np.float32) ** np.float32(1.0 / 3.0)))

    w0t, coef0 = _prep_layer01(np.asarray(wpn0), np.asarray(b0), np.asarray(act0), ls)
    w1t, coef1 = _prep_layer01(np.asarray(wpn1), np.asarray(b1), np.asarray(act1), ls)
    inw2 = wpn2.shape[0] // 2
    w2 = (np.asarray(wpn2)[:inw2].astype(np.float64)
          - np.asarray(wpn2)[inw2:].astype(np.float64)) * ls
    w2t = _tile_w(w2)
    b2c = np.ascontiguousarray(
        np.asarray(b2, dtype=np.float32).reshape(16, P).T)

    if "nc" not in _CACHE:
        _CACHE["nc"] = _build_program()
    nc = _CACHE["nc"]

    in_maps = []
    for c in range(N_CORES):
        xs = X[c * B_CORE:(c + 1) * B_CORE]            # [1024, 2048]
        xtc = np.ascontiguousarray(xs.T.astype(np.float16)).reshape(KT, P, B_CORE)
        in_maps.append({
            "xt": xtc,
            "w0t": w0t, "w1t": w1t, "w2t": w2t,
            "coef0": coef0, "coef1": coef1, "b2c": b2c,
        })

    res = run_bass_kernel_spmd(nc, in_maps, core_ids=list(range(N_CORES)))
    return np.concatenate([r["out"].T for r in res.results], axis=0)
